# revision 26
# baseline (speedup 1.0000x reference)
"""Trainium2 Bass kernel for the B-spline (KAN-style) layer.

Math: out[b,o] = sum_{i,k} B3_k(t(b,i)) * coeff[i,o,k], where t = tanh(x)
mapped to knot coordinates t = (tanh(x) - grid[0]) / h in (3, 8), and B3 are
cubic B-spline bases over uniform integer knots.

Key transformation: with a = t - 4 and L_m = relu(t-m)^3, the 8-dim spline
space on (3,8) (knots at 4,5,6,7) is exactly span{1, a, a^2, a^3, L4..L7}.
The CONSTANT basis function is rank-1 across the contraction, so it folds
into a per-output-feature bias added during PSUM eviction — the device
contraction needs only SEVEN planes per input feature (K = 7*512 = 3584
instead of 8*512), saving 1/8 of the matmul time. Per knot m:
    a_m = 2.5*tanh(x) + (5.5 - m)     (DVE tensor_scalar)
    s_m = Square(a_m)                 (ACT engine)
    c_m = s_m * a_m                   (DVE tensor_mul)
    L_m = max(c_m, 0)                 (DVE tensor_scalar_max)
m=4 contributes planes [a4, s4, c4, L4]; m=5,6,7 contribute only L_m (their
cubes are polynomial in a4 — folded into the a/s/c coefficients on the host).

Then a dense fp16 matmul: out[o,b] = sum_{(i,r)} C3[(i,r),o] * rho[(i,r),b]
with contraction K = 3584, run on the PE at the full fp16 rate
(216.7ns per 128x128x512 matmul), plus the bias during eviction.

Schedule notes (from trace analysis):
  - production is bh-major at half-batch width (FD=512), matching both the
    matmul passes and the HBM-bound arrival order of the input DMAs;
  - plane production is split ACT (squares) / DVE (affine+cube+relu) with
    order-only edges keeping the ACT queue in production order;
  - c3 coefficients ride the fast scalar HWDGE ring (chunk 0 split with the
    gpsimd ring so the first K-slices land before the plane chain finishes);
  - warm-up matmuls bridge the PE-busy window into the real stream so the
    HAM clock-gate is at 8/8 when it starts;
  - xt and out are stored tile-contiguous in DRAM; the final eviction is
    split across both copy engines and both DMA rings.

Sharding: data-parallel over batch (8192 -> 8 x 1024); coefficients replicated.
"""

from contextlib import ExitStack, nullcontext

import numpy as np

import concourse.bass as bass
import concourse.mybir as mybir
import concourse.tile as tile
from concourse.bass_utils import run_bass_kernel_spmd
from concourse.tile import add_dep_helper
from concourse.vector_clock import ScopedClock

F32 = mybir.dt.float32
F16 = mybir.dt.float16

N_CORES = 8
B_FULL = 8192
B_SHARD = B_FULL // N_CORES  # 1024
I_FEAT = 512
O_FEAT = 512
NPLANES = 7  # [a, a^2, a^3, L4, L5, L6, L7]
NCHUNK = I_FEAT // 128  # 4
ALU = mybir.AluOpType
AF = mybir.ActivationFunctionType

N_WARMUP = 7

# ---------------------------------------------------------------------------
# Workaround for walrus "Too many sync wait commands" on the TileContext final
# Drain: spread the accumulated semaphore waits across single-wait nofuse NOPs
# on the sync engine, then emit a bare drain + the usual barrier/cleanup.
_MAXW = 1


def _patched_drain_and_barrier(self, tick_clock, wait_clock):
    nc = self.nc
    probe = nc.sync.nop(nofuse=True)
    wait_clock.add_sem_waits(probe.ins, ScopedClock({None: tick_clock.global_clock}))
    si = probe.ins.sync_info
    waits = list(si.on_wait) if si and si.on_wait else []
    if len(waits) > _MAXW:
        si.on_wait = waits[:_MAXW]
        rest = waits[_MAXW:]
        while rest:
            chunk, rest = rest[:_MAXW], rest[_MAXW:]
            n2 = nc.sync.nop(nofuse=True)
            s2 = n2.ins.sync_info
            if s2 is None:
                n2.ins.sync_info = mybir.SyncInfo(on_wait=chunk, on_update=[])
            else:
                s2.on_wait = chunk
    nc.sync.drain()
    nc.all_engine_barrier()
    assert self.sems is not None
    popped = nc._tile_sem_poison_stack.pop()
    assert popped is self._sem_poison
    nc.clear_and_free_semaphores(list(self.sems.allocated().values()))


tile.TileContext._drain_and_barrier = _patched_drain_and_barrier


def _split_all_waits(nc: bass.Bass) -> None:
    """This image's walrus rejects instructions carrying more than one sync
    wait. Hoist all but the last wait of each instruction onto fresh NoOps on
    the same engine immediately before it (in-order issue makes this
    equivalent, merely slightly stronger synchronization)."""
    cnt = 0
    for f in nc.m.functions:
        for bb in f.blocks:
            out = []
            changed = False
            for inst in bb.instructions:
                si = inst.sync_info
                waits = list(si.on_wait) if si and si.on_wait else []
                if len(waits) > 1:
                    changed = True
                    for w in waits[:-1]:
                        nop = mybir.InstNoOp(name=f"waitsplit-{cnt}", ins=[], outs=[])
                        cnt += 1
                        nop.engine = inst.engine
                        nop.sync_info = mybir.SyncInfo(on_wait=[w], on_update=[])
                        out.append(nop)
                    si.on_wait = [waits[-1]]
                out.append(inst)
            if changed:
                bb.instructions = out


# ---------------------------------------------------------------------------


def _build_nc(t_scale: float, t_bias: float) -> bass.Bass:
    """Build the per-core Bass program.

    Per-core I/O (all tile-contiguous in DRAM):
      xt  : (4, 2, 128, 512) f16  x^T shard (chunk, batch-half, part, col)
      c3  : (4, 128, 3584) f16    folded coefficients [chunk, part, plane*512+o]
      bias: (128, 4) f32          per-output-feature bias [part, o-chunk]
      out : (4, 2, 128, 512) f32  output (o-chunk, batch-half, part, col)
    """
    nc = bass.Bass()
    xt = nc.declare_dram_parameter("xt", [NCHUNK, 2, 128, B_SHARD // 2], F16,
                                   isOutput=False)
    c3 = nc.declare_dram_parameter(
        "c3", [NCHUNK, 128, NPLANES * O_FEAT], F16, isOutput=False
    )
    bias = nc.declare_dram_parameter("bias", [128, NCHUNK], F32, isOutput=False)
    out = nc.declare_dram_parameter("out", [NCHUNK, 2, 128, B_SHARD // 2], F32,
                                    isOutput=True)

    with tile.TileContext(nc) as tc, ExitStack() as ctx:
        c3_pool = ctx.enter_context(tc.tile_pool(name="c3", bufs=1))
        xin_pool = ctx.enter_context(tc.tile_pool(name="xin", bufs=1))
        xn_pool = ctx.enter_context(tc.tile_pool(name="xn", bufs=1))
        sq_pool = ctx.enter_context(tc.tile_pool(name="sq", bufs=3))
        aff_pool = ctx.enter_context(tc.tile_pool(name="aff", bufs=3))
        cc_pool = ctx.enter_context(tc.tile_pool(name="ccs", bufs=3))
        rho_pool = ctx.enter_context(tc.tile_pool(name="rho", bufs=1))
        ps_pool = ctx.enter_context(
            tc.tile_pool(name="ps", bufs=1, space=bass.MemorySpace.PSUM)
        )
        ost_pool = ctx.enter_context(tc.tile_pool(name="ost", bufs=1))

        BHALF = B_SHARD // 2  # 512

        # Dummy 1-column activation with no deps: hoists the ~1.3us ACT
        # table load to kernel start, off the tanh critical path. All tiles
        # are kept 64B multiples — an odd allocation shifts later tiles to
        # 32-mod-64 addresses, slowing the PE moving-operand fetch ~20%.
        # PE warm-up source: memset FIRST on gpsimd so the warm-up matmuls
        # start as early as possible.
        wz = c3_pool.tile([128, 512], F16, tag="warmz")
        nc.gpsimd.memset(wz[:], 0.0)

        dummy = xn_pool.tile([128, 32], F16, tag="dummy")
        nc.gpsimd.memset(dummy[:], 0.0)
        nc.scalar.activation(dummy[:, :1], dummy[:, :1], AF.Tanh)

        # Per-o bias (the constant basis function, rank-1 across the
        # contraction): tiny, rides the sync ring behind xt00.
        bias_sb = c3_pool.tile([128, 16], F32, tag="bias")

        # c3 rides the scalar ring (measured ~2x the per-queue bandwidth of
        # the gpsimd ring and much more than the sync ring), with chunk 0
        # split across scalar+gpsimd so its first K-slices land before the
        # plane chain finishes.
        c3_sb = [
            c3_pool.tile([128, NPLANES * O_FEAT], F16, tag=f"c3_{c}",
                         name=f"c3sb_{c}")
            for c in range(NCHUNK)
        ]
        SPLITW = 3 * O_FEAT  # first three K-slices (a, s, c) on the scalar ring
        nc.scalar.dma_start(c3_sb[0][:, :SPLITW], c3[0][:, :SPLITW])
        nc.gpsimd.dma_start(c3_sb[0][:, SPLITW:], c3[0][:, SPLITW:])
        nc.scalar.dma_start(c3_sb[1][:], c3[1])
        nc.scalar.dma_start(c3_sb[2][:], c3[2])
        nc.scalar.dma_start(c3_sb[3][:], c3[3])

        # rho[h][c][r] planes ordered [a, s, c, L4, L5, L6, L7] (basis change
        # folded into c3 on the host). Production is bh-major at half-batch
        # width (FD=512), matching both the matmul passes and the HBM-bound
        # arrival order of the input DMAs.
        rho = [[[None] * NPLANES for _ in range(NCHUNK)] for _ in range(2)]

        # The ACT queue must execute tanh/squares strictly in production
        # order — the list scheduler otherwise hoists a later chunk's tanh
        # (gated on its xt DMA) ahead of the current chunk's squares,
        # starving the matmul stream for several us.
        act_chain = [None]

        def chain_act(inst):
            if act_chain[0] is not None:
                add_dep_helper(inst.ins, act_chain[0].ins, sync=False,
                               reason="ACT production order")
            act_chain[0] = inst
            return inst

        prev_hold = [None]

        def produce_tile(xn_t, c, h):
            first_tile = (c == 0 and h == 0)
            for mi, m in enumerate((4, 5, 6, 7)):
                beta = t_bias - m
                first = first_tile and mi == 0
                prio = tc.high_priority() if first else nullcontext()
                keep = (mi == 0)  # m=4: a, s, c are planes too
                with prio:
                    if keep:
                        a = rho_pool.tile([128, BHALF], F16,
                                          tag=f"rho{h}_{c}_0", name=f"a{c}{h}")
                    else:
                        a = aff_pool.tile([128, BHALF], F16, tag="ah",
                                          name=f"a{c}{h}{mi}")
                    a_i = nc.vector.tensor_scalar(
                        a[:], xn_t[:], t_scale, beta, ALU.mult, ALU.add
                    )
                    if first_tile and mi == 1 and prev_hold[0] is not None:
                        # Order-only edge: keep the scheduler from
                        # interleaving the next m's ops into the chain that
                        # gates the matmul stream.
                        add_dep_helper(a_i.ins, prev_hold[0].ins, sync=False,
                                       reason="first-chain order")
                    if keep:
                        sq = rho_pool.tile([128, BHALF], F16,
                                           tag=f"rho{h}_{c}_1", name=f"s{c}{h}")
                    else:
                        sq = sq_pool.tile([128, BHALF], F16, tag="sqh",
                                          name=f"sq{c}{h}{mi}")
                    if first:
                        # First unit: square on DVE — the ACT round-trip
                        # would sit on the first-matmul critical path.
                        nc.vector.tensor_mul(sq[:], a[:], a[:])
                    else:
                        chain_act(nc.scalar.activation(sq[:], a[:], AF.Square))
                    if keep:
                        cc = rho_pool.tile([128, BHALF], F16,
                                           tag=f"rho{h}_{c}_2", name=f"c{c}{h}")
                    else:
                        cc = cc_pool.tile([128, BHALF], F16, tag="cch",
                                          name=f"cc{c}{h}{mi}")
                    nc.vector.tensor_mul(cc[:], sq[:], a[:])
                    lp = rho_pool.tile([128, BHALF], F16,
                                       tag=f"rho{h}_{c}_{3 + mi}",
                                       name=f"lp{c}{h}{mi}")
                    lp_i = nc.vector.tensor_scalar_max(lp[:], cc[:], 0.0)
                    prev_hold[0] = lp_i
                if keep:
                    rho[h][c][0] = a[:]
                    rho[h][c][1] = sq[:]
                    rho[h][c][2] = cc[:]
                rho[h][c][3 + mi] = lp[:]

        # xt: the first tile rides the (otherwise idle) sync ring; the rest
        # ride gpsimd behind c3[0]'s tail, in exactly the order production
        # (and the HBM-bound arrival schedule) needs them.
        for h in range(2):
            for c in range(NCHUNK):
                xt_t = xin_pool.tile([128, BHALF], F16, tag=f"xt{c}{h}",
                                     name=f"xt{c}{h}")
                ring = nc.sync if (c == 0 and h == 0) else nc.gpsimd
                ring.dma_start(xt_t[:], xt[c][h])
                if c == 0 and h == 0:
                    nc.sync.dma_start(bias_sb[:, :NCHUNK], bias[:])
                xn_t = xn_pool.tile([128, BHALF], F16, tag=f"xn{c}{h}",
                                    name=f"xn{c}{h}")
                chain_act(nc.scalar.activation(xn_t[:], xt_t[:], AF.Tanh))
                produce_tile(xn_t, c, h)

        # Dense matmul: 8 PSUM tiles (o_chunk x b_half) accumulated over all
        # 28 (chunk, plane) K-slices. Pass h=0 is K-major (dense PE stream
        # consuming planes in production order); its PSUM eviction + output
        # DMA overlap pass h=1. Pass h=1 is o-major so each o-tile's
        # eviction + DMA trickle out during the remaining matmuls.
        ps = [
            [
                ps_pool.tile([128, 512], F32, tag=f"ps{o}_{h}", name=f"ps{o}_{h}")
                for h in range(2)
            ]
            for o in range(NCHUNK)
        ]
        NK = NCHUNK * NPLANES  # 28

        for _ in range(N_WARMUP):
            nc.tensor.matmul(
                ps[0][0][:64, :], wz[:, :64], wz[:], start=True, stop=True
            )

        def emit_copy_out(o, h):
            # Evictions add the per-o bias: ACT via Identity(in + bias_ap),
            # DVE via tensor_scalar_add with a per-partition AP. Copies
            # alternate ACT/DVE; output DMA triggers alternate the
            # sync/gpsimd HWDGE rings so the tail's trigger issue
            # parallelizes instead of queueing.
            bias_ap = bias_sb[:, o:o + 1]
            ot = ost_pool.tile([128, 512], F32, tag=f"ot{o}_{h}", name=f"ot{o}_{h}")
            if o == 3 and h == 1:
                # The very last eviction is fully exposed: split it across
                # both copy engines and both DMA rings to halve the
                # last-matmul -> final-drain chain.
                nc.scalar.activation(ot[:, :256], ps[o][h][:, :256],
                                     AF.Identity, bias=bias_ap)
                nc.vector.tensor_scalar_add(ot[:, 256:], ps[o][h][:, 256:],
                                            bias_ap)
                nc.sync.dma_start(out[o][h][:, :256], ot[:, :256])
                nc.gpsimd.dma_start(out[o][h][:, 256:], ot[:, 256:])
                return
            if o % 2 == 0:
                nc.scalar.activation(ot[:], ps[o][h][:], AF.Identity,
                                     bias=bias_ap)
            else:
                nc.vector.tensor_scalar_add(ot[:], ps[o][h][:], bias_ap)
            # In pass 2 the last tiles must trigger on the idle sync ring —
            # queueing behind gpsimd's earlier triggers costs ~0.7us of tail.
            even_ring = (o % 2 == 0) if h == 0 else (o % 2 == 1)
            eng = nc.sync if even_ring else nc.gpsimd
            eng.dma_start(out[o][h], ot[:])

        for kk in range(NK):
            c, r = divmod(kk, NPLANES)
            rt = rho[0][c][r]
            for o in range(4):
                lhsT = c3_sb[c][:, r * O_FEAT + o * 128 : r * O_FEAT + (o + 1) * 128]
                nc.tensor.matmul(
                    ps[o][0][:], lhsT, rt, start=(kk == 0), stop=(kk == NK - 1)
                )
        for o in range(4):
            emit_copy_out(o, 0)
        KTAIL = NK - 4
        for kk in range(KTAIL):
            c, r = divmod(kk, NPLANES)
            rt = rho[1][c][r]
            for o in range(4):
                lhsT = c3_sb[c][:, r * O_FEAT + o * 128 : r * O_FEAT + (o + 1) * 128]
                nc.tensor.matmul(
                    ps[o][1][:], lhsT, rt, start=(kk == 0), stop=False
                )
        for o in range(4):
            for kk in range(KTAIL, NK):
                c, r = divmod(kk, NPLANES)
                lhsT = c3_sb[c][:, r * O_FEAT + o * 128 : r * O_FEAT + (o + 1) * 128]
                nc.tensor.matmul(
                    ps[o][1][:],
                    lhsT,
                    rho[1][c][r],
                    start=False,
                    stop=(kk == NK - 1),
                )
            emit_copy_out(o, 1)
    _split_all_waits(nc)
    return nc


# Basis change onto [a, a^2, a^3, L4..L7] + constant, a = t - 4:
# B3_j = sum_m [(WL+WR)[j,m]/6] L_m - [WR[j,m]/6] c_m, c_m = (a - d_m)^3,
# d_m = m - 4. Expanding the cubes gives the gamma coefficients below.
_WL = np.array(
    [
        [0, 0, 0, 0],
        [0, 0, 0, 0],
        [0, 0, 0, 0],
        [0, 0, 0, 0],
        [1, -4, 6, -4],
        [0, 1, -4, 6],
        [0, 0, 1, -4],
        [0, 0, 0, 1],
    ],
    dtype=np.float64,
)
_WR = np.array(
    [
        [1, 0, 0, 0],
        [-4, 1, 0, 0],
        [6, -4, 1, 0],
        [-4, 6, -4, 1],
        [0, 0, 0, 0],
        [0, 0, 0, 0],
        [0, 0, 0, 0],
        [0, 0, 0, 0],
    ],
    dtype=np.float64,
)

_nc_cache: dict = {}


def _prepare(x: np.ndarray, coefficients: np.ndarray, grid: np.ndarray):
    x = np.asarray(x, dtype=np.float32)
    coefficients = np.asarray(coefficients, dtype=np.float32)
    grid = np.asarray(grid, dtype=np.float32)

    # Knot-coordinate transform t = (tanh(x) - grid[0]) / h (uniform grid).
    h = float(grid[-1] - grid[0]) / (len(grid) - 1)
    t_scale = 1.0 / h
    t_bias = -float(grid[0]) / h  # t = t_scale * xn + t_bias; here 2.5, 5.5

    key = (round(t_scale, 9), round(t_bias, 9))
    if key not in _nc_cache:
        _nc_cache[key] = _build_nc(t_scale, t_bias)
    nc = _nc_cache[key]

    # Host-side fold: W7[j, r] for planes [a, s, c, L4..L7] + bias gamma0.
    d = np.array([0.0, 1.0, 2.0, 3.0])
    g3 = -_WR.sum(axis=1) / 6.0
    g2 = (_WR * d).sum(axis=1) / 2.0
    g1 = -(_WR * d**2).sum(axis=1) / 2.0
    g0 = (_WR * d**3).sum(axis=1) / 6.0
    aL = (_WL + _WR) / 6.0
    w7 = np.column_stack([g1, g2, g3, aL[:, 0], aL[:, 1], aL[:, 2], aL[:, 3]])

    cf64 = coefficients.astype(np.float64)
    c3f = np.einsum("ioj,jr->iro", cf64, w7)
    c3_arr = np.ascontiguousarray(
        c3f.reshape(NCHUNK, 128, NPLANES, O_FEAT)
        .reshape(NCHUNK, 128, NPLANES * O_FEAT)
        .astype(np.float16)
    )
    bias_vec = np.einsum("ioj,j->o", cf64, g0)  # (512,)
    bias_arr = np.ascontiguousarray(
        bias_vec.reshape(NCHUNK, 128).T.astype(np.float32)
    )  # (128, 4): [part, o-chunk]

    # xt tile-contiguous: (chunk, half, part, col) per core.
    xt = x.T.astype(np.float16)  # (512, 8192)
    in_maps = []
    for core in range(N_CORES):
        xs = xt[:, core * B_SHARD : (core + 1) * B_SHARD]  # (512, 1024)
        xtc = np.ascontiguousarray(
            xs.reshape(NCHUNK, 128, 2, B_SHARD // 2).transpose(0, 2, 1, 3)
        )  # (4, 2, 128, 512)
        in_maps.append({"xt": xtc, "c3": c3_arr, "bias": bias_arr})
    return nc, in_maps


def kernel(x: np.ndarray, coefficients: np.ndarray, grid: np.ndarray) -> np.ndarray:
    nc, in_maps = _prepare(x, coefficients, grid)
    res = run_bass_kernel_spmd(nc, in_maps, list(range(N_CORES)), trace=False)
    outs = []
    for core in range(N_CORES):
        oc = res.results[core]["out"]  # (4, 2, 128, 512) = (o-chunk, half, part, col)
        out_t = oc.transpose(0, 2, 1, 3).reshape(O_FEAT, B_SHARD)  # (512, 1024)
        outs.append(out_t)
    out_full = np.concatenate(outs, axis=1)  # (512, 8192)
    return np.ascontiguousarray(out_full.T).astype(np.float32)


# revision 27
# speedup vs baseline: 1.0148x; 1.0148x over previous
"""Trainium2 Bass kernel for the B-spline (KAN-style) layer.

Math: out[b,o] = sum_{i,k} B3_k(t(b,i)) * coeff[i,o,k], where t = tanh(x)
mapped to knot coordinates t = (tanh(x) - grid[0]) / h in (3, 8), and B3 are
cubic B-spline bases over uniform integer knots.

Key transformation: with a = t - 4 and L_m = relu(t-m)^3, the 8-dim spline
space on (3,8) (knots at 4,5,6,7) is exactly span{1, a, a^2, a^3, L4..L7}.
The CONSTANT basis function is rank-1 across the contraction, so it folds
into a per-output-feature bias added during PSUM eviction — the device
contraction needs only SEVEN planes per input feature (K = 7*512 = 3584
instead of 8*512), saving 1/8 of the matmul time. Per knot m:
    a_m = 2.5*tanh(x) + (5.5 - m)     (DVE tensor_scalar)
    s_m = Square(a_m)                 (ACT engine)
    c_m = s_m * a_m                   (DVE tensor_mul)
    L_m = max(c_m, 0)                 (DVE tensor_scalar_max)
m=4 contributes planes [a4, s4, c4, L4]; m=5,6,7 contribute only L_m (their
cubes are polynomial in a4 — folded into the a/s/c coefficients on the host).

Then a dense fp16 matmul: out[o,b] = sum_{(i,r)} C3[(i,r),o] * rho[(i,r),b]
with contraction K = 3584, run on the PE at the full fp16 rate
(216.7ns per 128x128x512 matmul), plus the bias during eviction.

Schedule notes (from trace analysis):
  - production is bh-major at half-batch width (FD=512), matching both the
    matmul passes and the HBM-bound arrival order of the input DMAs;
  - plane production is split ACT (squares) / DVE (affine+cube+relu) with
    order-only edges keeping the ACT queue in production order;
  - c3 coefficients ride the fast scalar HWDGE ring (chunk 0 split with the
    gpsimd ring so the first K-slices land before the plane chain finishes);
  - warm-up matmuls bridge the PE-busy window into the real stream so the
    HAM clock-gate is at 8/8 when it starts;
  - xt and out are stored tile-contiguous in DRAM; the final eviction is
    split across both copy engines and both DMA rings.

Sharding: data-parallel over batch (8192 -> 8 x 1024); coefficients replicated.
"""

from contextlib import ExitStack, nullcontext

import numpy as np

import concourse.bass as bass
import concourse.mybir as mybir
import concourse.tile as tile
from concourse.bass_utils import run_bass_kernel_spmd
from concourse.tile import add_dep_helper
from concourse.vector_clock import ScopedClock

F32 = mybir.dt.float32
F16 = mybir.dt.float16

N_CORES = 8
B_FULL = 8192
B_SHARD = B_FULL // N_CORES  # 1024
I_FEAT = 512
O_FEAT = 512
NPLANES = 7  # [a, a^2, a^3, L4, L5, L6, L7]
NCHUNK = I_FEAT // 128  # 4
ALU = mybir.AluOpType
AF = mybir.ActivationFunctionType

N_WARMUP = 7

# ---------------------------------------------------------------------------
# Workaround for walrus "Too many sync wait commands" on the TileContext final
# Drain: spread the accumulated semaphore waits across single-wait nofuse NOPs
# on the sync engine, then emit a bare drain + the usual barrier/cleanup.
_MAXW = 1


def _patched_drain_and_barrier(self, tick_clock, wait_clock):
    nc = self.nc
    probe = nc.sync.nop(nofuse=True)
    wait_clock.add_sem_waits(probe.ins, ScopedClock({None: tick_clock.global_clock}))
    si = probe.ins.sync_info
    waits = list(si.on_wait) if si and si.on_wait else []
    if len(waits) > _MAXW:
        si.on_wait = waits[:_MAXW]
        rest = waits[_MAXW:]
        while rest:
            chunk, rest = rest[:_MAXW], rest[_MAXW:]
            n2 = nc.sync.nop(nofuse=True)
            s2 = n2.ins.sync_info
            if s2 is None:
                n2.ins.sync_info = mybir.SyncInfo(on_wait=chunk, on_update=[])
            else:
                s2.on_wait = chunk
    nc.sync.drain()
    nc.all_engine_barrier()
    assert self.sems is not None
    popped = nc._tile_sem_poison_stack.pop()
    assert popped is self._sem_poison
    nc.clear_and_free_semaphores(list(self.sems.allocated().values()))


tile.TileContext._drain_and_barrier = _patched_drain_and_barrier


def _split_all_waits(nc: bass.Bass) -> None:
    """This image's walrus rejects instructions carrying more than one sync
    wait. Hoist all but the last wait of each instruction onto fresh NoOps on
    the same engine immediately before it (in-order issue makes this
    equivalent, merely slightly stronger synchronization)."""
    cnt = 0
    for f in nc.m.functions:
        for bb in f.blocks:
            out = []
            changed = False
            for inst in bb.instructions:
                si = inst.sync_info
                waits = list(si.on_wait) if si and si.on_wait else []
                if len(waits) > 1:
                    changed = True
                    for w in waits[:-1]:
                        nop = mybir.InstNoOp(name=f"waitsplit-{cnt}", ins=[], outs=[])
                        cnt += 1
                        nop.engine = inst.engine
                        nop.sync_info = mybir.SyncInfo(on_wait=[w], on_update=[])
                        out.append(nop)
                    si.on_wait = [waits[-1]]
                out.append(inst)
            if changed:
                bb.instructions = out


# ---------------------------------------------------------------------------


def _build_nc(t_scale: float, t_bias: float) -> bass.Bass:
    """Build the per-core Bass program.

    Per-core I/O (all tile-contiguous in DRAM):
      xt  : (4, 2, 128, 512) f16  x^T shard (chunk, batch-half, part, col)
      c3  : (4, 128, 3584) f16    folded coefficients [chunk, part, plane*512+o]
      bias: (128, 4) f32          per-output-feature bias [part, o-chunk]
      out : (4, 2, 128, 512) f32  output (o-chunk, batch-half, part, col)
    """
    nc = bass.Bass()
    xt = nc.declare_dram_parameter("xt", [NCHUNK, 2, 128, B_SHARD // 2], F16,
                                   isOutput=False)
    c3 = nc.declare_dram_parameter(
        "c3", [NCHUNK, 128, NPLANES * O_FEAT], F16, isOutput=False
    )
    bias = nc.declare_dram_parameter("bias", [128, NCHUNK], F32, isOutput=False)
    out = nc.declare_dram_parameter("out", [NCHUNK, 2, 128, B_SHARD // 2], F32,
                                    isOutput=True)

    with tile.TileContext(nc) as tc, ExitStack() as ctx:
        c3_pool = ctx.enter_context(tc.tile_pool(name="c3", bufs=1))
        xin_pool = ctx.enter_context(tc.tile_pool(name="xin", bufs=1))
        xn_pool = ctx.enter_context(tc.tile_pool(name="xn", bufs=1))
        sq_pool = ctx.enter_context(tc.tile_pool(name="sq", bufs=3))
        aff_pool = ctx.enter_context(tc.tile_pool(name="aff", bufs=3))
        cc_pool = ctx.enter_context(tc.tile_pool(name="ccs", bufs=3))
        rho_pool = ctx.enter_context(tc.tile_pool(name="rho", bufs=1))
        ps_pool = ctx.enter_context(
            tc.tile_pool(name="ps", bufs=1, space=bass.MemorySpace.PSUM)
        )
        ost_pool = ctx.enter_context(tc.tile_pool(name="ost", bufs=1))

        BHALF = B_SHARD // 2  # 512

        # Dummy 1-column activation with no deps: hoists the ~1.3us ACT
        # table load to kernel start, off the tanh critical path. All tiles
        # are kept 64B multiples — an odd allocation shifts later tiles to
        # 32-mod-64 addresses, slowing the PE moving-operand fetch ~20%.
        # PE warm-up source: memset FIRST on gpsimd so the warm-up matmuls
        # start as early as possible.
        wz = c3_pool.tile([128, 512], F16, tag="warmz")
        nc.gpsimd.memset(wz[:], 0.0)

        dummy = xn_pool.tile([128, 32], F16, tag="dummy")
        nc.gpsimd.memset(dummy[:], 0.0)
        nc.scalar.activation(dummy[:, :1], dummy[:, :1], AF.Tanh)

        # Per-o bias (the constant basis function, rank-1 across the
        # contraction): tiny, rides the sync ring behind xt00.
        bias_sb = c3_pool.tile([128, 16], F32, tag="bias")

        # c3 rides the scalar ring (measured ~2x the per-queue bandwidth of
        # the gpsimd ring and much more than the sync ring), with chunk 0
        # split across scalar+gpsimd so its first K-slices land before the
        # plane chain finishes.
        c3_sb = [
            c3_pool.tile([128, NPLANES * O_FEAT], F16, tag=f"c3_{c}",
                         name=f"c3sb_{c}")
            for c in range(NCHUNK)
        ]
        SPLITW = 3 * O_FEAT  # first three K-slices (a, s, c) on the scalar ring
        nc.scalar.dma_start(c3_sb[0][:, :SPLITW], c3[0][:, :SPLITW])
        nc.gpsimd.dma_start(c3_sb[0][:, SPLITW:], c3[0][:, SPLITW:])
        nc.scalar.dma_start(c3_sb[1][:], c3[1])
        nc.scalar.dma_start(c3_sb[2][:], c3[2])
        nc.scalar.dma_start(c3_sb[3][:], c3[3])

        # rho[h][c][r] planes ordered [a, s, c, L4, L5, L6, L7] (basis change
        # folded into c3 on the host). Production is bh-major at half-batch
        # width (FD=512), matching both the matmul passes and the HBM-bound
        # arrival order of the input DMAs.
        rho = [[[None] * NPLANES for _ in range(NCHUNK)] for _ in range(2)]

        # The ACT queue must execute tanh/squares strictly in production
        # order — the list scheduler otherwise hoists a later chunk's tanh
        # (gated on its xt DMA) ahead of the current chunk's squares,
        # starving the matmul stream for several us.
        act_chain = [None]

        def chain_act(inst):
            if act_chain[0] is not None:
                add_dep_helper(inst.ins, act_chain[0].ins, sync=False,
                               reason="ACT production order")
            act_chain[0] = inst
            return inst

        prev_hold = [None]

        def produce_tile(xn_t, c, h):
            first_tile = (c == 0 and h == 0)
            for mi, m in enumerate((4, 5, 6, 7)):
                beta = t_bias - m
                first = first_tile and mi == 0
                prio = tc.high_priority() if first else nullcontext()
                keep = (mi == 0)  # m=4: a, s, c are planes too
                with prio:
                    if keep:
                        a = rho_pool.tile([128, BHALF], F16,
                                          tag=f"rho{h}_{c}_0", name=f"a{c}{h}")
                    else:
                        a = aff_pool.tile([128, BHALF], F16, tag="ah",
                                          name=f"a{c}{h}{mi}")
                    a_i = nc.vector.tensor_scalar(
                        a[:], xn_t[:], t_scale, beta, ALU.mult, ALU.add
                    )
                    if first_tile and mi >= 1 and prev_hold[0] is not None:
                        # Order-only edge: keep the scheduler from
                        # interleaving the next m's ops into the chain that
                        # gates the matmul stream.
                        add_dep_helper(a_i.ins, prev_hold[0].ins, sync=False,
                                       reason="first-chain order")
                    if keep:
                        sq = rho_pool.tile([128, BHALF], F16,
                                           tag=f"rho{h}_{c}_1", name=f"s{c}{h}")
                    else:
                        sq = sq_pool.tile([128, BHALF], F16, tag="sqh",
                                          name=f"sq{c}{h}{mi}")
                    if first:
                        # First unit: square on DVE — the ACT round-trip
                        # would sit on the first-matmul critical path.
                        nc.vector.tensor_mul(sq[:], a[:], a[:])
                    else:
                        chain_act(nc.scalar.activation(sq[:], a[:], AF.Square))
                    if keep:
                        cc = rho_pool.tile([128, BHALF], F16,
                                           tag=f"rho{h}_{c}_2", name=f"c{c}{h}")
                    else:
                        cc = cc_pool.tile([128, BHALF], F16, tag="cch",
                                          name=f"cc{c}{h}{mi}")
                    nc.vector.tensor_mul(cc[:], sq[:], a[:])
                    lp = rho_pool.tile([128, BHALF], F16,
                                       tag=f"rho{h}_{c}_{3 + mi}",
                                       name=f"lp{c}{h}{mi}")
                    lp_i = nc.vector.tensor_scalar_max(lp[:], cc[:], 0.0)
                    prev_hold[0] = lp_i
                if keep:
                    rho[h][c][0] = a[:]
                    rho[h][c][1] = sq[:]
                    rho[h][c][2] = cc[:]
                rho[h][c][3 + mi] = lp[:]

        # xt: the first tile rides the (otherwise idle) sync ring; the rest
        # ride gpsimd behind c3[0]'s tail, in exactly the order production
        # (and the HBM-bound arrival schedule) needs them.
        for h in range(2):
            for c in range(NCHUNK):
                xt_t = xin_pool.tile([128, BHALF], F16, tag=f"xt{c}{h}",
                                     name=f"xt{c}{h}")
                ring = nc.sync if (c == 0 and h == 0) else nc.gpsimd
                ring.dma_start(xt_t[:], xt[c][h])
                if c == 0 and h == 0:
                    nc.sync.dma_start(bias_sb[:, :NCHUNK], bias[:])
                xn_t = xn_pool.tile([128, BHALF], F16, tag=f"xn{c}{h}",
                                    name=f"xn{c}{h}")
                chain_act(nc.scalar.activation(xn_t[:], xt_t[:], AF.Tanh))
                produce_tile(xn_t, c, h)

        # Dense matmul: 8 PSUM tiles (o_chunk x b_half) accumulated over all
        # 28 (chunk, plane) K-slices. Pass h=0 is K-major (dense PE stream
        # consuming planes in production order); its PSUM eviction + output
        # DMA overlap pass h=1. Pass h=1 is o-major so each o-tile's
        # eviction + DMA trickle out during the remaining matmuls.
        ps = [
            [
                ps_pool.tile([128, 512], F32, tag=f"ps{o}_{h}", name=f"ps{o}_{h}")
                for h in range(2)
            ]
            for o in range(NCHUNK)
        ]
        NK = NCHUNK * NPLANES  # 28

        for _ in range(N_WARMUP):
            nc.tensor.matmul(
                ps[0][0][:64, :], wz[:, :64], wz[:], start=True, stop=True
            )

        def emit_copy_out(o, h):
            # Evictions add the per-o bias: ACT via Identity(in + bias_ap),
            # DVE via tensor_scalar_add with a per-partition AP. Copies
            # alternate ACT/DVE; output DMA triggers alternate the
            # sync/gpsimd HWDGE rings so the tail's trigger issue
            # parallelizes instead of queueing.
            bias_ap = bias_sb[:, o:o + 1]
            ot = ost_pool.tile([128, 512], F32, tag=f"ot{o}_{h}", name=f"ot{o}_{h}")
            if o == 3 and h == 1:
                # The very last eviction is fully exposed: split it across
                # both copy engines and both DMA rings to halve the
                # last-matmul -> final-drain chain.
                nc.scalar.activation(ot[:, :256], ps[o][h][:, :256],
                                     AF.Identity, bias=bias_ap)
                nc.vector.tensor_scalar_add(ot[:, 256:], ps[o][h][:, 256:],
                                            bias_ap)
                nc.sync.dma_start(out[o][h][:, :256], ot[:, :256])
                nc.gpsimd.dma_start(out[o][h][:, 256:], ot[:, 256:])
                return
            if o % 2 == 0:
                nc.scalar.activation(ot[:], ps[o][h][:], AF.Identity,
                                     bias=bias_ap)
            else:
                nc.vector.tensor_scalar_add(ot[:], ps[o][h][:], bias_ap)
            # In pass 2 the last tiles must trigger on the idle sync ring —
            # queueing behind gpsimd's earlier triggers costs ~0.7us of tail.
            even_ring = (o % 2 == 0) if h == 0 else (o % 2 == 1)
            eng = nc.sync if even_ring else nc.gpsimd
            eng.dma_start(out[o][h], ot[:])

        for kk in range(NK):
            c, r = divmod(kk, NPLANES)
            rt = rho[0][c][r]
            for o in range(4):
                lhsT = c3_sb[c][:, r * O_FEAT + o * 128 : r * O_FEAT + (o + 1) * 128]
                nc.tensor.matmul(
                    ps[o][0][:], lhsT, rt, start=(kk == 0), stop=(kk == NK - 1)
                )
        for o in range(4):
            emit_copy_out(o, 0)
        KTAIL = NK - 4
        for kk in range(KTAIL):
            c, r = divmod(kk, NPLANES)
            rt = rho[1][c][r]
            for o in range(4):
                lhsT = c3_sb[c][:, r * O_FEAT + o * 128 : r * O_FEAT + (o + 1) * 128]
                nc.tensor.matmul(
                    ps[o][1][:], lhsT, rt, start=(kk == 0), stop=False
                )
        for o in range(4):
            for kk in range(KTAIL, NK):
                c, r = divmod(kk, NPLANES)
                lhsT = c3_sb[c][:, r * O_FEAT + o * 128 : r * O_FEAT + (o + 1) * 128]
                nc.tensor.matmul(
                    ps[o][1][:],
                    lhsT,
                    rho[1][c][r],
                    start=False,
                    stop=(kk == NK - 1),
                )
            emit_copy_out(o, 1)
    _split_all_waits(nc)
    return nc


# Basis change onto [a, a^2, a^3, L4..L7] + constant, a = t - 4:
# B3_j = sum_m [(WL+WR)[j,m]/6] L_m - [WR[j,m]/6] c_m, c_m = (a - d_m)^3,
# d_m = m - 4. Expanding the cubes gives the gamma coefficients below.
_WL = np.array(
    [
        [0, 0, 0, 0],
        [0, 0, 0, 0],
        [0, 0, 0, 0],
        [0, 0, 0, 0],
        [1, -4, 6, -4],
        [0, 1, -4, 6],
        [0, 0, 1, -4],
        [0, 0, 0, 1],
    ],
    dtype=np.float64,
)
_WR = np.array(
    [
        [1, 0, 0, 0],
        [-4, 1, 0, 0],
        [6, -4, 1, 0],
        [-4, 6, -4, 1],
        [0, 0, 0, 0],
        [0, 0, 0, 0],
        [0, 0, 0, 0],
        [0, 0, 0, 0],
    ],
    dtype=np.float64,
)

_nc_cache: dict = {}


def _prepare(x: np.ndarray, coefficients: np.ndarray, grid: np.ndarray):
    x = np.asarray(x, dtype=np.float32)
    coefficients = np.asarray(coefficients, dtype=np.float32)
    grid = np.asarray(grid, dtype=np.float32)

    # Knot-coordinate transform t = (tanh(x) - grid[0]) / h (uniform grid).
    h = float(grid[-1] - grid[0]) / (len(grid) - 1)
    t_scale = 1.0 / h
    t_bias = -float(grid[0]) / h  # t = t_scale * xn + t_bias; here 2.5, 5.5

    key = (round(t_scale, 9), round(t_bias, 9))
    if key not in _nc_cache:
        _nc_cache[key] = _build_nc(t_scale, t_bias)
    nc = _nc_cache[key]

    # Host-side fold: W7[j, r] for planes [a, s, c, L4..L7] + bias gamma0.
    d = np.array([0.0, 1.0, 2.0, 3.0])
    g3 = -_WR.sum(axis=1) / 6.0
    g2 = (_WR * d).sum(axis=1) / 2.0
    g1 = -(_WR * d**2).sum(axis=1) / 2.0
    g0 = (_WR * d**3).sum(axis=1) / 6.0
    aL = (_WL + _WR) / 6.0
    w7 = np.column_stack([g1, g2, g3, aL[:, 0], aL[:, 1], aL[:, 2], aL[:, 3]])

    cf64 = coefficients.astype(np.float64)
    c3f = np.einsum("ioj,jr->iro", cf64, w7)
    c3_arr = np.ascontiguousarray(
        c3f.reshape(NCHUNK, 128, NPLANES, O_FEAT)
        .reshape(NCHUNK, 128, NPLANES * O_FEAT)
        .astype(np.float16)
    )
    bias_vec = np.einsum("ioj,j->o", cf64, g0)  # (512,)
    bias_arr = np.ascontiguousarray(
        bias_vec.reshape(NCHUNK, 128).T.astype(np.float32)
    )  # (128, 4): [part, o-chunk]

    # xt tile-contiguous: (chunk, half, part, col) per core.
    xt = x.T.astype(np.float16)  # (512, 8192)
    in_maps = []
    for core in range(N_CORES):
        xs = xt[:, core * B_SHARD : (core + 1) * B_SHARD]  # (512, 1024)
        xtc = np.ascontiguousarray(
            xs.reshape(NCHUNK, 128, 2, B_SHARD // 2).transpose(0, 2, 1, 3)
        )  # (4, 2, 128, 512)
        in_maps.append({"xt": xtc, "c3": c3_arr, "bias": bias_arr})
    return nc, in_maps


def kernel(x: np.ndarray, coefficients: np.ndarray, grid: np.ndarray) -> np.ndarray:
    nc, in_maps = _prepare(x, coefficients, grid)
    res = run_bass_kernel_spmd(nc, in_maps, list(range(N_CORES)), trace=False)
    outs = []
    for core in range(N_CORES):
        oc = res.results[core]["out"]  # (4, 2, 128, 512) = (o-chunk, half, part, col)
        out_t = oc.transpose(0, 2, 1, 3).reshape(O_FEAT, B_SHARD)  # (512, 1024)
        outs.append(out_t)
    out_full = np.concatenate(outs, axis=1)  # (512, 8192)
    return np.ascontiguousarray(out_full.T).astype(np.float32)
